# revision 31
# baseline (speedup 1.0000x reference)
"""TRN2 Bass kernel for nn_MultiHeadAttention (B=4, S=2048, D=1024, H=16, DH=64).

Sharding (8 cores): core c -> batch b = c//2, head-half hh = c%2 (8 heads each).
Host: out[b] = core(2b) partial + core(2b+1) partial.

v4: the f32r attention baseline with three structural cuts.

Quantization noise in attention does NOT average out (ctx is a softmax-
weighted mean: signal and noise shrink by the same sqrt(Neff)), so every
quantized factor on the q/k/at/v path passes its per-element relative error
straight to the output; plain fp8 anywhere there costs 4-8% (measured).
Scores and PV therefore stay at >=bf16 and stream S^2/128 columns each at
1 cycle/row -- the irreducible PE core (~109us each).  What CAN be cut:

  - Projections run as residual-fp8 DoubleRow: the host ships x and W as
    fp8 (hi, lo) pairs (lo = fp8(a - fp8(a)), ~0.15% effective error) and
    each q/k/v tile accumulates three chunk-paired DoubleRow passes
    (xh.wh + xh.wl + xl.wh) at 0.5 cyc/row: 25% fewer PE cycles than f32r
    at ~0.26% projection error.
  - The softmax-denominator reciprocal broadcast moves off the PE/DVE onto
    the idle GPSIMD (Pool) engine via partition_broadcast.
  - at / V_aug are bf16 (0.11% rms): same 1 cyc/row matmul rate, half the
    SBUF, and exp->bf16 keeps the ACT stream at its 1038ns/[128,1024] floor.

Structure (single TileContext; phases overlap via data deps): v projection
with pair-0 q/k first, then per-head attention with pairs 1-3 projections
pumped two steps per (head, qb).  Scores land transposed (sT[sk, sq] =
kT.T @ qT) so dh sits on partitions; V_aug carries a ones column per head
so the PV matmul also yields softmax denominators (ctx row 64).  This
walrus build accepts only ONE sync-wait per instruction, so extra waits
are split into single-wait NoOps (legalize_waits)."""

import sys

if "/opt/trn_rl_repo" not in sys.path:
    sys.path.insert(0, "/opt/trn_rl_repo")

import numpy as np
import ml_dtypes

import concourse.bass as bass
import concourse.mybir as mybir
import concourse.tile as tile
from concourse.bass_utils import run_bass_kernel_spmd

F32 = mybir.dt.float32
F32R = mybir.dt.float32r
BF16 = mybir.dt.bfloat16
F8 = mybir.dt.float8e4
NPF8 = ml_dtypes.float8_e4m3
DR = mybir.MatmulPerfMode.DoubleRow
EXP = mybir.ActivationFunctionType.Exp

B, S_FULL, D, H = 4, 2048, 1024, 16
DH = 64
NCORES = 8
XSCALE = 4.0   # host prescale of x before fp8 split (keeps residuals normal)
WSCALE = 16.0  # host prescale of W ~ N(0, 1/1024) out of the fp8 subnormal band
QSCALE = XSCALE * WSCALE  # q/k/v arrive scaled by this; folded into exp scale
EXP_SCALE = 0.125 / (QSCALE * QSCALE)  # and the 64.0 V_aug ones column
CSCALE = 32.0   # ctx prescale before fp8 split (via the `ones` stationary)
OSCALE = 1.0 / (CSCALE * WSCALE)  # undone in the output bias-add


def legalize_waits(nc, max_waits=1):
    """Split >max_waits sync-waits per instruction into single-wait NoOps on
    the same engine, placed immediately before (per-engine order preserved)."""
    n = 0
    for fn in nc.m.functions:
        for blk in fn.blocks:
            out = []
            for inst in blk.instructions:
                si = inst.sync_info
                if si is not None and len(si.on_wait) > max_waits:
                    waits = list(si.on_wait)
                    for w in waits[:-max_waits]:
                        nop = mybir.InstNoOp(
                            name=f"WSPLIT-{n}", ins=[], outs=[], engine=inst.engine
                        )
                        n += 1
                        nop.sync_info = mybir.SyncInfo(on_wait=[w], on_update=[])
                        out.append(nop)
                    inst.sync_info = mybir.SyncInfo(
                        on_wait=waits[-max_waits:], on_update=list(si.on_update)
                    )
                out.append(inst)
            blk.instructions[:] = out
    return n


def _bcast_ap(src_ap, parts=128):
    """Partition-broadcast a [1, N] AP to [parts, N] via a step-0 dim."""
    return bass.AP(
        tensor=src_ap.tensor,
        offset=src_ap.offset,
        ap=[[0, parts], list(src_ap.ap[-1])],
    )


def _pair_ap(src_ap, i_stride):
    """Insert a DoubleRow K-tile dim: [K, N] -> [K, 2, N] with the second
    tile at +i_stride elements (0 = same data twice)."""
    return bass.AP(
        tensor=src_ap.tensor,
        offset=src_ap.offset,
        ap=[list(src_ap.ap[0]), [i_stride, 2], list(src_ap.ap[-1])],
    )


def build_nc(S=S_FULL, legalize=True):
    NQB = S // 1024  # 1024-wide sq blocks
    NST = S // 128   # sk tiles
    NSB = S // 512   # 512-wide s blocks (projection granularity)
    nc = bass.Bass()
    xh_d = nc.dram_tensor("xth", [D, S], F8, kind="ExternalInput")
    xl_d = nc.dram_tensor("xtl", [D, S], F8, kind="ExternalInput")
    w_d = {}
    for w in ("wq", "wk", "wv"):
        for part in ("h", "l"):
            w_d[w + part] = nc.dram_tensor(w + part, [128, 4096], F8,
                                           kind="ExternalInput")
    woh_d = nc.dram_tensor("woh", [128, 4096], F8, kind="ExternalInput")
    wol_d = nc.dram_tensor("wol", [128, 4096], F8, kind="ExternalInput")
    bqk_d = nc.dram_tensor("bqk", [128, 8], F32, kind="ExternalInput")
    # packed broadcast constants: [bv(512) | bo(1024)]
    cpk_d = nc.dram_tensor("cpk", [1, 1536], F32, kind="ExternalInput")
    ones_d = nc.dram_tensor("ones", [1, 64], F32R, kind="ExternalInput")
    vinit_d = nc.dram_tensor("vinit", [1, NST * 520], BF16, kind="ExternalInput")
    out_d = nc.dram_tensor("out", [S, 1024], F32, kind="ExternalOutput")

    with tile.TileContext(nc) as tc, nc.allow_low_precision(
        reason="residual-fp8 projections, bf16 attention weights"
    ):
        with tc.tile_pool(name="persist", bufs=1) as pp, \
             tc.tile_pool(name="psP", bufs=2, space="PSUM") as psP, \
             tc.tile_pool(name="psS", bufs=2, space="PSUM") as psS, \
             tc.tile_pool(name="psX", bufs=2, space="PSUM") as psX:
            qT = pp.tile([128, 4 * S], F32R)
            kT = pp.tile([128, 4 * S], F32R)
            vall = pp.tile([128, NST * 520], BF16)  # per s-tile: 8 heads x 65
            wt = {}
            for w in ("wq", "wk", "wv"):
                for part in ("h", "l"):
                    wt[w + part] = pp.tile([128, 4096], F8, tag=f"t{w}{part}",
                                           name=f"t{w}{part}")
            woh = pp.tile([128, 4096], F8)
            wol = pp.tile([128, 4096], F8)
            bqk = pp.tile([128, 8], F32)
            cpk = pp.tile([128, 1536], F32)
            ones = pp.tile([1, 64], F32R)
            bv_b = cpk[:, 0:512]
            bo_b = cpk[:, 512:1536]
            # x stays RESIDENT in fp8 hi/lo pairs (4MB total): one load,
            # no per-pass reloads.  Chunk-major: col = ch*S + s.
            xfh = pp.tile([128, 8 * S], F8)
            xfl = pp.tile([128, 8 * S], F8)

            def load_xsb(sb):
                """One strided DMA per residual half for s-block sb, on
                separate DGE queues (descriptor-gen parallelism)."""
                for eng, t, d in ((nc.sync, xfh, xh_d), (nc.scalar, xfl, xl_d)):
                    eng.dma_start(
                        out=t[:, :].rearrange("p (ch s) -> p ch s", s=S)
                        [:, :, sb * 512:(sb + 1) * 512],
                        in_=bass.AP(tensor=d, offset=sb * 512,
                                    ap=[[S, 128], [128 * S, 8], [1, 512]]),
                    )

            def qk_group(wname, dstT, bcol, p, sb):
                """q/k projection for head-pair p over s-block sb: three
                chunk-paired residual DoubleRow passes."""
                ps_q = psP.tile([128, 512], F32, tag="pp", name="ps_q")
                sets = ((wt[wname + "h"], xfh), (wt[wname + "l"], xfh),
                        (wt[wname + "h"], xfl))
                for ck in range(2):
                    for si, (wm, xm) in enumerate(sets):
                        for cp in range(4):
                            nc.tensor.matmul(
                                ps_q[:, ck * 256:(ck + 1) * 256],
                                _pair_ap(wm[:, cp * 1024 + p * 128:
                                            cp * 1024 + p * 128 + 128], 512),
                                _pair_ap(xm[:, 2 * cp * S + sb * 512
                                            + ck * 256:
                                            2 * cp * S + sb * 512
                                            + ck * 256 + 256], S),
                                start=(si == 0 and cp == 0),
                                stop=(si == 2 and cp == 3),
                                perf_mode=DR,
                            )
                nc.vector.tensor_scalar_add(
                    dstT[:, p * S + sb * 512: p * S + (sb + 1) * 512],
                    ps_q,
                    bqk[:, bcol + p: bcol + p + 1],
                )

            # ---- projection sweep 1: pair 0 q/k + all of v ----
            if True:
                # DMA issue order: first-needed bytes first.  Pair-0 columns
                # of wq/wk (cols ch*512..+128 of each 512-block) land first
                # so the first qk_group starts after ~1.5MB, not 3MB.
                load_xsb(0)
                for n in ("wkh", "wkl", "wqh", "wql"):
                    nc.scalar.dma_start(
                        out=wt[n][:, :].rearrange("p (ch s) -> p ch s", ch=8)
                        [:, :, 0:128],
                        in_=bass.AP(tensor=w_d[n], offset=0,
                                    ap=[[4096, 128], [512, 8], [1, 128]]),
                    )
                nc.sync.dma_start(out=bqk, in_=bqk_d[:, :])
                load_xsb(1)
                for n in ("wvh", "wvl"):
                    nc.scalar.dma_start(out=wt[n], in_=w_d[n][:, :])
                nc.sync.dma_start(out=cpk, in_=_bcast_ap(cpk_d[:, :]))
                nc.sync.dma_start(out=ones, in_=ones_d[:, :])
                # V_aug template (QSCALE in each head's 65th col)
                nc.sync.dma_start(out=vall, in_=_bcast_ap(vinit_d[:, :]))
                load_xsb(2)
                load_xsb(3)
                for n in ("wkh", "wkl", "wqh", "wql"):
                    nc.scalar.dma_start(
                        out=wt[n][:, :].rearrange("p (ch s) -> p ch s", ch=8)
                        [:, :, 128:512],
                        in_=bass.AP(tensor=w_d[n], offset=128,
                                    ap=[[4096, 128], [512, 8], [1, 384]]),
                    )

                for sb in range(NSB):
                    qk_group("wk", kT, 4, 0, sb)
                for sb in range(2):
                    qk_group("wq", qT, 0, 0, sb)
                vsets = ((xfh, wt["wvh"]), (xfh, wt["wvl"]),
                         (xfl, wt["wvh"]))
                for sb in range(NSB):
                    for t4 in range(4):
                        st = sb * 4 + t4
                        ps_v = psP.tile([128, 512], F32, tag="pp", name="ps_v")
                        for ck in range(2):
                            for si, (xm, wm) in enumerate(vsets):
                                for cp in range(4):
                                    nc.tensor.matmul(
                                        ps_v[:, ck * 256:(ck + 1) * 256],
                                        _pair_ap(xm[:, 2 * cp * S + sb * 512
                                                    + t4 * 128:
                                                    2 * cp * S + sb * 512
                                                    + t4 * 128 + 128], S),
                                        _pair_ap(wm[:, cp * 1024 + ck * 256:
                                                    cp * 1024 + ck * 256 + 256],
                                                 512),
                                        start=(si == 0 and cp == 0),
                                        stop=(si == 2 and cp == 3),
                                        perf_mode=DR,
                                    )
                        dst = vall[:, st * 520:(st + 1) * 520].rearrange(
                            "p (h e) -> p h e", e=65
                        )[:, :, 0:64]
                        nc.vector.tensor_add(
                            dst,
                            ps_v.rearrange("p (h e) -> p h e", e=64),
                            bv_b.rearrange("p (h e) -> p h e", e=64),
                        )

            # ---- attention; pairs 1-3 projections interleaved ----
            # qb outer: once the qb=0 half of every head is done (unit 8),
            # its out-projection tiles interleave with qb=1 attention, using
            # the psP banks that pass2 projections (done by unit 6) vacate.
            nc.sync.dma_start(out=woh, in_=woh_d[:, :])  # needed from unit 8
            nc.sync.dma_start(out=wol, in_=wol_d[:, :])
            with tc.tile_pool(name="bc", bufs=1) as bc:
              ctxT = bc.tile([128, 8 * S], F8)  # hi cols [0,4S), lo [4S,8S)
              with tc.tile_pool(name="at", bufs=5) as atp, \
                   tc.tile_pool(name="cot", bufs=3) as cot, \
                   tc.tile_pool(name="sm", bufs=2) as sm:

                def pass2_front():
                    for p in (1, 2, 3):
                        for sb in range(NSB):
                            qk_group("wk", kT, 4, p, sb)
                            yield
                        for sb in range(2):
                            qk_group("wq", qT, 0, p, sb)
                            yield

                def pass2_defer():
                    for p in range(4):
                        for sb in (2, 3):
                            qk_group("wq", qT, 0, p, sb)
                            yield

                pass2 = pass2_front()
                p2def = pass2_defer()

                OSETS = ((0, woh), (0, wol), (4 * S, woh))

                def emit_out_mms(ps, t, half, n0, nw):
                    """Residual DR out-projection chunk [128, nw] at
                    column n0 of the given half: 3 sets x chunk-paired.
                    Chunk loop outermost so each PSUM zero-region sees one
                    closed accumulation group at a time."""
                    for nc_ in range(nw // 256):
                        for si, (co, wm) in enumerate(OSETS):
                            for i in range(2):
                                c = half * 512 + nc_ * 256
                                nc.tensor.matmul(
                                    ps[:, n0 + nc_ * 256: n0 + nc_ * 256 + 256],
                                    _pair_ap(ctxT[:, co + 2 * i * S + t * 128:
                                                  co + 2 * i * S + t * 128
                                                  + 128], S),
                                    _pair_ap(wm[:, 2 * i * 1024 + c:
                                                2 * i * 1024 + c + 256], 1024),
                                    start=(si == 0 and i == 0),
                                    stop=(si == 2 and i == 1),
                                    perf_mode=DR,
                                )

                def out_tile(t):
                    """Output-projection rows t*128..+128 in two halves
                    (psP-sized PSUM), residual-fp8 DoubleRow."""
                    for half in range(2):
                        ps_oh = psP.tile([128, 512], F32, tag="pp",
                                         name="ps_oh")
                        emit_out_mms(ps_oh, t, half, 0, 512)
                        ot = cot.tile([128, 512], F32, tag="ot")
                        nc.vector.scalar_tensor_tensor(
                            ot, ps_oh, OSCALE,
                            bo_b[:, half * 512:(half + 1) * 512],
                            op0=mybir.AluOpType.mult,
                            op1=mybir.AluOpType.add,
                        )
                        nc.sync.dma_start(
                            out=out_d[t * 128:(t + 1) * 128,
                                      half * 512:(half + 1) * 512],
                            in_=ot,
                        )

                for qb in range(NQB):
                    for h in range(8):
                        unit = qb * 8 + h
                        p = h // 2
                        r0 = 64 * (h % 2)
                        if qb == 1:
                            out_tile(h)  # qb=0 rows overlap qb=1 attention
                        ps_c = [psX.tile([65, 512], F32, tag="pctx",
                                         name=f"ps_c{_i}")
                                for _i in range(2)]
                        def emit_pv(st, at):
                            for half in range(2):
                                nc.tensor.matmul(
                                    ps_c[half],
                                    vall[:, st * 520 + h * 65:
                                         st * 520 + (h + 1) * 65],
                                    at[:, half * 512:(half + 1) * 512],
                                    start=(st == 0),
                                    stop=(st == NST - 1),
                                )

                        pv_pending = None
                        for st in range(NST):
                            # front pumps: 3/unit through units 0-5 (pair
                            # p+1's k + qb0-q land by unit 2(p+1)); deferred
                            # qb1-q pumps ride the ACT slack of units 6-11
                            # (pair p's qb1 q needed by unit 8+2p).
                            if unit < 6 and st in (9, 11, 13):
                                next(pass2, None)
                            elif 6 <= unit < 8 and st in (6, 12):
                                next(p2def, None)
                            elif 8 <= unit < 12 and st == 10:
                                next(p2def, None)
                            ps_s = psS.tile([128, 1024], F32, tag="ps")
                            for half in range(2):
                                nc.tensor.matmul(
                                    ps_s[:, half * 512:(half + 1) * 512],
                                    kT[r0:r0 + 64,
                                       p * S + st * 128: p * S + (st + 1) * 128],
                                    qT[r0:r0 + 64,
                                       p * S + qb * 1024 + half * 512:
                                       p * S + qb * 1024 + (half + 1) * 512],
                                    start=True,
                                    stop=True,
                                )
                            at = atp.tile([128, 1024], BF16, tag="at")
                            nc.scalar.activation(at, ps_s, EXP, scale=EXP_SCALE)
                            if pv_pending is not None:
                                emit_pv(*pv_pending)
                            pv_pending = (st, at)
                        emit_pv(*pv_pending)
                        for half in range(2):
                            rsum = sm.tile([1, 512], F32R, tag="rsum")
                            nc.vector.reciprocal(rsum, ps_c[half][64:65, :])
                            ps_b = psP.tile([64, 512], F32, tag="pp",
                                            name="ps_b")
                            nc.tensor.matmul(ps_b, ones, rsum,
                                             start=True, stop=True)
                            rb = sm.tile([64, 512], F32, tag="rb")
                            nc.vector.tensor_copy(rb, ps_b)
                            c0 = p * S + qb * 1024 + half * 512
                            cff = sm.tile([128, 512], F32R, tag="cf",
                                          name="cff")
                            cf = cff[r0:r0 + 64, :]
                            nc.vector.tensor_mul(cf, ps_c[half][0:64, :], rb)
                            hi = ctxT[r0:r0 + 64, c0:c0 + 512]
                            nc.vector.tensor_copy(hi, cf)
                            nc.vector.tensor_sub(
                                ctxT[r0:r0 + 64, 4 * S + c0: 4 * S + c0 + 512],
                                cf, hi,
                            )
                for _ in pass2:
                    pass
                for _ in p2def:
                    pass

                # ---- qb=1 out-projection tail: psS is free now, so use
                # full [128, 1024] PSUM tiles (deeper pipelining than the
                # psP halves used during the overlap phase) ----
                for t in range(NST // 2, NST):
                    ps_o = psS.tile([128, 1024], F32, tag="ps", name="ps_o")
                    for half in range(2):
                        emit_out_mms(ps_o, t, half, half * 512, 512)
                    ot = cot.tile([128, 1024], F32, tag="ot2", name="ot2")
                    nc.vector.scalar_tensor_tensor(
                        ot, ps_o, OSCALE, bo_b,
                        op0=mybir.AluOpType.mult,
                        op1=mybir.AluOpType.add,
                    )
                    nc.sync.dma_start(out=out_d[t * 128:(t + 1) * 128, :],
                                      in_=ot)

    if legalize:
        legalize_waits(nc)
    return nc


def pack_core_inputs(c, x, Wq, bq, Wk, bk, Wv, bv, Wo, bo, S=S_FULL):
    """Pack full-model inputs into core c's device tensors."""
    b = c // 2
    hh = c % 2
    hs = slice(hh * 8, hh * 8 + 8)

    def pack_w(W):  # [8, D, DH] -> [128, 4096]: free = chunk*512 + (h*64+dh)
        W2 = np.transpose(W, (1, 0, 2)).reshape(D, 512)      # [d, h*dh]
        return np.ascontiguousarray(
            np.transpose(W2.reshape(8, 128, 512), (1, 0, 2)).reshape(128, 4096)
        )

    def split8(a):
        hi = a.astype(NPF8)
        lo = (a - hi.astype(np.float32)).astype(NPF8)
        return hi, lo

    xT = np.ascontiguousarray(x[b].T)                         # [D, S]
    xh, xl = split8(XSCALE * xT.astype(np.float32))
    wqh, wql = split8(WSCALE * pack_w(Wq[hs]))
    wkh, wkl = split8(WSCALE * pack_w(Wk[hs]))
    wvh, wvl = split8(WSCALE * pack_w(Wv[hs]))
    # Wo rows for this half's features: [512, 1024] -> [128, 4*1024]
    Wr = Wo[hh * 512:(hh + 1) * 512]
    wo = np.ascontiguousarray(
        np.transpose(Wr.reshape(4, 128, 1024), (1, 0, 2)).reshape(128, 4096)
    )
    woh, wol = split8(WSCALE * wo)
    bqk = QSCALE * np.concatenate(
        [bq[hs].reshape(4, 128).T, bk[hs].reshape(4, 128).T], axis=1
    )                                                         # [128, 8]
    bvp = QSCALE * bv[hs].reshape(1, 512)
    bop = (0.5 * bo).reshape(1, 1024)
    NST = S // 128
    vinit = np.zeros((1, NST * 520), dtype=np.float32)
    # ones column = QSCALE so ps_c row 64 = QSCALE*sum(at): its reciprocal
    # normalizes the QSCALE-scaled v in one step.
    vinit[0, 64::65] = QSCALE
    cpk = np.concatenate([
        bvp.ravel().astype(np.float32),
        bop.ravel().astype(np.float32),
    ]).reshape(1, 1536)
    return {
        "vinit": vinit.astype(ml_dtypes.bfloat16),
        "cpk": cpk,
        "ones": np.full((1, 64), CSCALE, dtype=np.float32),
        "xth": xh, "xtl": xl,
        "wqh": wqh, "wql": wql,
        "wkh": wkh, "wkl": wkl,
        "wvh": wvh, "wvl": wvl,
        "woh": woh, "wol": wol,
        "bqk": np.ascontiguousarray(bqk).astype(np.float32),
    }


_NC_CACHE = {}


def _get_nc(S=S_FULL):
    if S not in _NC_CACHE:
        _NC_CACHE[S] = build_nc(S)
    return _NC_CACHE[S]


def kernel(x, Wq, bq, Wk, bk, Wv, bv, Wo, bo, _trace=False):
    x, Wq, bq, Wk, bk, Wv, bv, Wo, bo = (
        np.asarray(a, dtype=np.float32) for a in (x, Wq, bq, Wk, bk, Wv, bv, Wo, bo)
    )
    nc = _get_nc()
    in_maps = [
        pack_core_inputs(c, x, Wq, bq, Wk, bk, Wv, bv, Wo, bo) for c in range(NCORES)
    ]
    res = run_bass_kernel_spmd(nc, in_maps, list(range(NCORES)), trace=_trace)
    out = np.empty((B, S_FULL, D), dtype=np.float32)
    for b in range(B):
        out[b] = res.results[2 * b]["out"] + res.results[2 * b + 1]["out"]
    if _trace:
        kernel.last_results = res
    return out


# revision 42
# speedup vs baseline: 1.0399x; 1.0399x over previous
"""TRN2 Bass kernel for nn_MultiHeadAttention (B=4, S=2048, D=1024, H=16, DH=64).

Sharding (8 cores): core c -> batch b = c//2, head-half hh = c%2 (8 heads each).
Host: out[b] = core(2b) partial + core(2b+1) partial.

Quantization noise in attention does NOT average out (ctx is a softmax-
weighted mean: signal and noise shrink by the same sqrt(Neff)), so every
quantized factor on the q/k/at/v path passes its per-element relative error
straight to the output; plain fp8 anywhere there costs 4-8% (measured).
Scores and PV therefore stay at >=f32r/bf16 and stream S^2/128 columns each
at 1 cycle/row -- the irreducible PE core (~109us each) -- and the exp over
8 heads x S^2 scores paces the ACT engine at ~266us.  What IS cut vs the
f32r baseline:

  - Projections (q/k/v) and the output projection run as residual-fp8
    DoubleRow: the host ships x (scaled x4), W (x16) and Wo (x16) as fp8
    (hi, lo) pairs (lo = fp8(a - fp8(a)), ~0.15% effective error); each
    tile accumulates three chunk-paired DoubleRow passes (ah.bh + ah.bl +
    al.bh) at 0.5 cyc/row: 25% fewer PE cycles than f32r at ~0.26% error.
    ctx is likewise stored as x32-scaled fp8 pairs (scale carried by the
    `ones` reciprocal-broadcast stationary), and all scales cancel through
    the exp scale (0.125/QSCALE^2), the QSCALE ones column of V_aug, and
    the output bias-add (x OSCALE).
  - x stays RESIDENT in SBUF as fp8 pairs (4MB): one load, no per-pass
    reloads; DMAs issue on both the SP and ACT HWDGE queues in deadline
    order (descriptor-gen is the scarce resource; GPSIMD SWDGE hangs the
    device in this build, as does InstPartitionBroadcast in walrus).
  - at / V_aug are bf16 (0.11% rms): same 1 cyc/row matmul rate, half the
    SBUF; exp->bf16 keeps the ACT stream at its 1038ns/[128,1024] floor.

Schedule (single TileContext; phases overlap via data deps): pair-0 k/q
projections in DMA-arrival order with v interleaved (v for the last two
s-blocks lands just-in-time inside unit 0); attention runs qb-OUTER over
16 (qb, head) units so the qb=0 output-projection rows overlap qb=1
attention in the psP banks that pass2 pumps vacate; pairs 1-3 k + qb0-q
project 3 pumps/unit through units 0-5, with each pair's qb1-q deferred to
units 6-11; per-unit softmax normalization is deferred into the next
unit's st=1 slot.  Scores land transposed (sT[sk, sq] = kT.T @ qT) so dh
sits on partitions; V_aug carries a QSCALE column per head so the PV
matmul also yields softmax denominators (ctx row 64).  This walrus build
accepts only ONE sync-wait per instruction, so extra waits are split into
single-wait NoOps (legalize_waits)."""

import sys

if "/opt/trn_rl_repo" not in sys.path:
    sys.path.insert(0, "/opt/trn_rl_repo")

import numpy as np
import ml_dtypes

import concourse.bass as bass
import concourse.mybir as mybir
import concourse.tile as tile
from concourse.bass_utils import run_bass_kernel_spmd

F32 = mybir.dt.float32
F32R = mybir.dt.float32r
BF16 = mybir.dt.bfloat16
F8 = mybir.dt.float8e4
NPF8 = ml_dtypes.float8_e4m3
DR = mybir.MatmulPerfMode.DoubleRow
EXP = mybir.ActivationFunctionType.Exp

B, S_FULL, D, H = 4, 2048, 1024, 16
DH = 64
NCORES = 8
XSCALE = 4.0   # host prescale of x before fp8 split (keeps residuals normal)
WSCALE = 16.0  # host prescale of W ~ N(0, 1/1024) out of the fp8 subnormal band
QSCALE = XSCALE * WSCALE  # q/k/v arrive scaled by this; folded into exp scale
EXP_SCALE = 0.125 / (QSCALE * QSCALE)  # and the 64.0 V_aug ones column
CSCALE = 32.0   # ctx prescale before fp8 split (via the `ones` stationary)
OSCALE = 1.0 / (CSCALE * WSCALE)  # undone in the output bias-add


def legalize_waits(nc, max_waits=1):
    """Split >max_waits sync-waits per instruction into single-wait NoOps on
    the same engine, placed immediately before (per-engine order preserved)."""
    n = 0
    for fn in nc.m.functions:
        for blk in fn.blocks:
            out = []
            for inst in blk.instructions:
                si = inst.sync_info
                if si is not None and len(si.on_wait) > max_waits:
                    waits = list(si.on_wait)
                    for w in waits[:-max_waits]:
                        nop = mybir.InstNoOp(
                            name=f"WSPLIT-{n}", ins=[], outs=[], engine=inst.engine
                        )
                        n += 1
                        nop.sync_info = mybir.SyncInfo(on_wait=[w], on_update=[])
                        out.append(nop)
                    inst.sync_info = mybir.SyncInfo(
                        on_wait=waits[-max_waits:], on_update=list(si.on_update)
                    )
                out.append(inst)
            blk.instructions[:] = out
    return n


def _bcast_ap(src_ap, parts=128):
    """Partition-broadcast a [1, N] AP to [parts, N] via a step-0 dim."""
    return bass.AP(
        tensor=src_ap.tensor,
        offset=src_ap.offset,
        ap=[[0, parts], list(src_ap.ap[-1])],
    )


def _pair_ap(src_ap, i_stride):
    """Insert a DoubleRow K-tile dim: [K, N] -> [K, 2, N] with the second
    tile at +i_stride elements (0 = same data twice)."""
    return bass.AP(
        tensor=src_ap.tensor,
        offset=src_ap.offset,
        ap=[list(src_ap.ap[0]), [i_stride, 2], list(src_ap.ap[-1])],
    )


def build_nc(S=S_FULL, legalize=True):
    NQB = S // 1024  # 1024-wide sq blocks
    NST = S // 128   # sk tiles
    NSB = S // 512   # 512-wide s blocks (projection granularity)
    nc = bass.Bass()
    xh_d = nc.dram_tensor("xth", [D, S], F8, kind="ExternalInput")
    xl_d = nc.dram_tensor("xtl", [D, S], F8, kind="ExternalInput")
    w_d = {}
    for w in ("wq", "wk", "wv"):
        for part in ("h", "l"):
            w_d[w + part] = nc.dram_tensor(w + part, [128, 4096], F8,
                                           kind="ExternalInput")
    woh_d = nc.dram_tensor("woh", [128, 4096], F8, kind="ExternalInput")
    wol_d = nc.dram_tensor("wol", [128, 4096], F8, kind="ExternalInput")
    bqk_d = nc.dram_tensor("bqk", [128, 8], F32, kind="ExternalInput")
    # packed broadcast constants: [bv(512) | bo(1024)]
    cpk_d = nc.dram_tensor("cpk", [1, 1536], F32, kind="ExternalInput")
    ones_d = nc.dram_tensor("ones", [1, 64], F32R, kind="ExternalInput")
    vinit_d = nc.dram_tensor("vinit", [1, NST * 520], BF16, kind="ExternalInput")
    out_d = nc.dram_tensor("out", [S, 1024], F32, kind="ExternalOutput")

    with tile.TileContext(nc) as tc, nc.allow_low_precision(
        reason="residual-fp8 projections, bf16 attention weights"
    ):
        with tc.tile_pool(name="persist", bufs=1) as pp, \
             tc.tile_pool(name="psP", bufs=2, space="PSUM") as psP, \
             tc.tile_pool(name="psS", bufs=2, space="PSUM") as psS, \
             tc.tile_pool(name="psX", bufs=2, space="PSUM") as psX:
            qT = pp.tile([128, 4 * S], F32R)
            kT = pp.tile([128, 4 * S], F32R)
            vall = pp.tile([128, NST * 520], BF16)  # per s-tile: 8 heads x 65
            wt = {}
            for w in ("wq", "wk", "wv"):
                for part in ("h", "l"):
                    wt[w + part] = pp.tile([128, 4096], F8, tag=f"t{w}{part}",
                                           name=f"t{w}{part}")
            woh = pp.tile([128, 4096], F8)
            wol = pp.tile([128, 4096], F8)
            bqk = pp.tile([128, 8], F32)
            cpk = pp.tile([128, 1536], F32)
            ones = pp.tile([1, 64], F32R)
            bv_b = cpk[:, 0:512]
            bo_b = cpk[:, 512:1536]
            # x stays RESIDENT in fp8 hi/lo pairs (4MB total): one load,
            # no per-pass reloads.  Chunk-major: col = ch*S + s.
            xfh = pp.tile([128, 8 * S], F8)
            xfl = pp.tile([128, 8 * S], F8)

            def load_xsb(sb):
                """One strided DMA per residual half for s-block sb, on
                separate DGE queues (descriptor-gen parallelism)."""
                for eng, t, d in ((nc.sync, xfh, xh_d), (nc.scalar, xfl, xl_d)):
                    eng.dma_start(
                        out=t[:, :].rearrange("p (ch s) -> p ch s", s=S)
                        [:, :, sb * 512:(sb + 1) * 512],
                        in_=bass.AP(tensor=d, offset=sb * 512,
                                    ap=[[S, 128], [128 * S, 8], [1, 512]]),
                    )

            def qk_group(wname, dstT, bcol, p, sb):
                """q/k projection for head-pair p over s-block sb: three
                chunk-paired residual DoubleRow passes."""
                ps_q = psP.tile([128, 512], F32, tag="pp", name="ps_q")
                sets = ((wt[wname + "h"], xfh), (wt[wname + "l"], xfh),
                        (wt[wname + "h"], xfl))
                for ck in range(2):
                    for si, (wm, xm) in enumerate(sets):
                        for cp in range(4):
                            nc.tensor.matmul(
                                ps_q[:, ck * 256:(ck + 1) * 256],
                                _pair_ap(wm[:, cp * 1024 + p * 128:
                                            cp * 1024 + p * 128 + 128], 512),
                                _pair_ap(xm[:, 2 * cp * S + sb * 512
                                            + ck * 256:
                                            2 * cp * S + sb * 512
                                            + ck * 256 + 256], S),
                                start=(si == 0 and cp == 0),
                                stop=(si == 2 and cp == 3),
                                perf_mode=DR,
                            )
                nc.vector.tensor_scalar_add(
                    dstT[:, p * S + sb * 512: p * S + (sb + 1) * 512],
                    ps_q,
                    bqk[:, bcol + p: bcol + p + 1],
                )

            # ---- projection sweep 1: pair 0 q/k + all of v ----
            if True:
                # DMA issue order: first-needed bytes first.  Pair-0 columns
                # of wq/wk (cols ch*512..+128 of each 512-block) land first
                # so the first qk_group starts after ~1.5MB, not 3MB.
                load_xsb(0)
                for n in ("wkh", "wkl", "wqh", "wql"):
                    nc.scalar.dma_start(
                        out=wt[n][:, :].rearrange("p (ch s) -> p ch s", ch=8)
                        [:, :, 0:128],
                        in_=bass.AP(tensor=w_d[n], offset=0,
                                    ap=[[4096, 128], [512, 8], [1, 128]]),
                    )
                nc.sync.dma_start(out=bqk, in_=bqk_d[:, :])
                load_xsb(1)
                for n in ("wvh", "wvl"):
                    nc.scalar.dma_start(out=wt[n], in_=w_d[n][:, :])
                load_xsb(2)
                load_xsb(3)
                nc.sync.dma_start(out=cpk, in_=_bcast_ap(cpk_d[:, :]))
                nc.sync.dma_start(out=ones, in_=ones_d[:, :])
                # V_aug template (QSCALE in each head's 65th col)
                nc.sync.dma_start(out=vall, in_=_bcast_ap(vinit_d[:, :]))
                for n in ("wkh", "wkl", "wqh", "wql"):
                    nc.scalar.dma_start(
                        out=wt[n][:, :].rearrange("p (ch s) -> p ch s", ch=8)
                        [:, :, 128:512],
                        in_=bass.AP(tensor=w_d[n], offset=128,
                                    ap=[[4096, 128], [512, 8], [1, 384]]),
                    )

                # compute ordered by DMA arrival: k/q for early s-blocks,
                # then v interleaved with late x blocks
                vsets = ((xfh, wt["wvh"]), (xfh, wt["wvl"]),
                         (xfl, wt["wvh"]))

                def v_group(sb, t4):
                    st = sb * 4 + t4
                    ps_v = psP.tile([128, 512], F32, tag="pp", name="ps_v")
                    for ck in range(2):
                        for si, (xm, wm) in enumerate(vsets):
                            for cp in range(4):
                                nc.tensor.matmul(
                                    ps_v[:, ck * 256:(ck + 1) * 256],
                                    _pair_ap(xm[:, 2 * cp * S + sb * 512
                                                + t4 * 128:
                                                2 * cp * S + sb * 512
                                                + t4 * 128 + 128], S),
                                    _pair_ap(wm[:, cp * 1024 + ck * 256:
                                                cp * 1024 + ck * 256 + 256],
                                             512),
                                    start=(si == 0 and cp == 0),
                                    stop=(si == 2 and cp == 3),
                                    perf_mode=DR,
                                )
                    dst = vall[:, st * 520:(st + 1) * 520].rearrange(
                        "p (h e) -> p h e", e=65
                    )[:, :, 0:64]
                    nc.vector.tensor_add(
                        dst,
                        ps_v.rearrange("p (h e) -> p h e", e=64),
                        bv_b.rearrange("p (h e) -> p h e", e=64),
                    )

                # k/q for pair 0 in DMA-arrival order; v sb0/1 here, v sb2/3
                # just-in-time inside unit 0's st loop
                sweep = [("k", 0), ("q", 0), ("k", 1), ("q", 1), ("v", 0),
                         ("k", 2), ("v", 1), ("k", 3)]
                for kind, sb in sweep:
                    if kind == "k":
                        qk_group("wk", kT, 4, 0, sb)
                    elif kind == "q":
                        qk_group("wq", qT, 0, 0, sb)
                    else:
                        for t4 in range(4):
                            v_group(sb, t4)

            # ---- attention; pairs 1-3 projections interleaved ----
            # qb outer: once the qb=0 half of every head is done (unit 8),
            # its out-projection tiles interleave with qb=1 attention, using
            # the psP banks that pass2 projections (done by unit 6) vacate.
            nc.sync.dma_start(out=woh, in_=woh_d[:, :])  # needed from unit 8
            nc.sync.dma_start(out=wol, in_=wol_d[:, :])
            with tc.tile_pool(name="bc", bufs=1) as bc:
              ctxT = bc.tile([128, 8 * S], F8)  # hi cols [0,4S), lo [4S,8S)
              with tc.tile_pool(name="at", bufs=5) as atp, \
                   tc.tile_pool(name="cot", bufs=3) as cot, \
                   tc.tile_pool(name="sm", bufs=2) as sm:

                def pass2_front():
                    for p in (1, 2, 3):
                        for sb in range(NSB):
                            qk_group("wk", kT, 4, p, sb)
                            yield
                        for sb in range(2):
                            qk_group("wq", qT, 0, p, sb)
                            yield

                def pass2_defer():
                    for p in range(4):
                        for sb in (2, 3):
                            qk_group("wq", qT, 0, p, sb)
                            yield

                pass2 = pass2_front()
                p2def = pass2_defer()

                OSETS = ((0, woh), (0, wol), (4 * S, woh))

                def emit_out_mms(ps, t, half, n0, nw):
                    """Residual DR out-projection chunk [128, nw] at
                    column n0 of the given half: 3 sets x chunk-paired.
                    Chunk loop outermost so each PSUM zero-region sees one
                    closed accumulation group at a time."""
                    for nc_ in range(nw // 256):
                        for si, (co, wm) in enumerate(OSETS):
                            for i in range(2):
                                c = half * 512 + nc_ * 256
                                nc.tensor.matmul(
                                    ps[:, n0 + nc_ * 256: n0 + nc_ * 256 + 256],
                                    _pair_ap(ctxT[:, co + 2 * i * S + t * 128:
                                                  co + 2 * i * S + t * 128
                                                  + 128], S),
                                    _pair_ap(wm[:, 2 * i * 1024 + c:
                                                2 * i * 1024 + c + 256], 1024),
                                    start=(si == 0 and i == 0),
                                    stop=(si == 2 and i == 1),
                                    perf_mode=DR,
                                )

                def out_tile_half(t, half):
                    """Output-projection rows t*128..+128, one 512 half
                    (psP-sized PSUM), residual-fp8 DoubleRow."""
                    ps_oh = psP.tile([128, 512], F32, tag="pp",
                                     name="ps_oh")
                    emit_out_mms(ps_oh, t, half, 0, 512)
                    ot = cot.tile([128, 512], F32, tag="ot")
                    nc.vector.scalar_tensor_tensor(
                        ot, ps_oh, OSCALE,
                        bo_b[:, half * 512:(half + 1) * 512],
                        op0=mybir.AluOpType.mult,
                        op1=mybir.AluOpType.add,
                    )
                    nc.sync.dma_start(
                        out=out_d[t * 128:(t + 1) * 128,
                                  half * 512:(half + 1) * 512],
                        in_=ot,
                    )

                drain_prev = [None]
                for qb in range(NQB):
                    for h in range(8):
                        unit = qb * 8 + h
                        p = h // 2
                        r0 = 64 * (h % 2)
                        ps_c = [psX.tile([65, 512], F32, tag="pctx",
                                         name=f"ps_c{_i}")
                                for _i in range(2)]
                        def emit_pv(st, at):
                            for half in range(2):
                                nc.tensor.matmul(
                                    ps_c[half],
                                    vall[:, st * 520 + h * 65:
                                         st * 520 + (h + 1) * 65],
                                    at[:, half * 512:(half + 1) * 512],
                                    start=(st == 0),
                                    stop=(st == NST - 1),
                                )

                        pv_pending = None
                        for st in range(NST):
                            # front pumps: 3/unit through units 0-5 (pair
                            # p+1's k + qb0-q land by unit 2(p+1)); deferred
                            # qb1-q pumps ride the ACT slack of units 6-11
                            # (pair p's qb1 q needed by unit 8+2p).
                            if unit < 6 and st in (9, 11, 13):
                                next(pass2, None)
                            elif 6 <= unit < 8 and st in (6, 12):
                                next(p2def, None)
                            elif 8 <= unit < 12 and st == 10:
                                next(p2def, None)
                            if qb == 1 and st == 3:
                                out_tile_half(h, 0)  # qb=0 rows overlap qb=1
                            elif qb == 1 and st == 13:
                                out_tile_half(h, 1)
                            if unit == 0 and st >= 8:
                                v_group(st // 4, st % 4)  # JIT v for sb2/3
                            ps_s = psS.tile([128, 1024], F32, tag="ps")
                            for half in range(2):
                                nc.tensor.matmul(
                                    ps_s[:, half * 512:(half + 1) * 512],
                                    kT[r0:r0 + 64,
                                       p * S + st * 128: p * S + (st + 1) * 128],
                                    qT[r0:r0 + 64,
                                       p * S + qb * 1024 + half * 512:
                                       p * S + qb * 1024 + (half + 1) * 512],
                                    start=True,
                                    stop=True,
                                )
                            at = atp.tile([128, 1024], BF16, tag="at")
                            nc.scalar.activation(at, ps_s, EXP, scale=EXP_SCALE)
                            if st == 1 and drain_prev[0] is not None:
                                drain_prev[0]()
                                drain_prev[0] = None
                            if pv_pending is not None:
                                emit_pv(*pv_pending)
                            pv_pending = (st, at)

                        emit_pv(*pv_pending)

                        # Defer this unit's normalize drain into the next
                        # unit's st=1 slot: the PE boundary tail would
                        # otherwise starve the ACT exp stream.
                        def mk_drain(ps_c=ps_c, p=p, r0=r0, qb=qb):
                            def d():
                                for half in range(2):
                                    rsum = sm.tile([1, 512], F32R, tag="rsum")
                                    nc.vector.reciprocal(
                                        rsum, ps_c[half][64:65, :])
                                    ps_b = psP.tile([64, 512], F32, tag="pp",
                                                    name="ps_b")
                                    nc.tensor.matmul(ps_b, ones, rsum,
                                                     start=True, stop=True)
                                    rb = sm.tile([64, 512], F32, tag="rb")
                                    nc.vector.tensor_copy(rb, ps_b)
                                    c0 = p * S + qb * 1024 + half * 512
                                    cff = sm.tile([128, 512], F32R, tag="cf",
                                                  name="cff")
                                    cf = cff[r0:r0 + 64, :]
                                    nc.vector.tensor_mul(
                                        cf, ps_c[half][0:64, :], rb)
                                    hi = ctxT[r0:r0 + 64, c0:c0 + 512]
                                    nc.vector.tensor_copy(hi, cf)
                                    nc.vector.tensor_sub(
                                        ctxT[r0:r0 + 64,
                                             4 * S + c0: 4 * S + c0 + 512],
                                        cf, hi,
                                    )
                            return d

                        drain_prev[0] = mk_drain()
                if drain_prev[0] is not None:
                    drain_prev[0]()
                    drain_prev[0] = None
                for _ in pass2:
                    pass
                for _ in p2def:
                    pass

                # ---- qb=1 out-projection tail: psS is free now, so use
                # full [128, 1024] PSUM tiles (deeper pipelining than the
                # psP halves used during the overlap phase) ----
                for t in range(NST // 2, NST):
                    ps_o = psS.tile([128, 1024], F32, tag="ps", name="ps_o")
                    for half in range(2):
                        emit_out_mms(ps_o, t, half, half * 512, 512)
                    ot = cot.tile([128, 1024], F32, tag="ot2", name="ot2")
                    nc.vector.scalar_tensor_tensor(
                        ot, ps_o, OSCALE, bo_b,
                        op0=mybir.AluOpType.mult,
                        op1=mybir.AluOpType.add,
                    )
                    nc.sync.dma_start(out=out_d[t * 128:(t + 1) * 128, :],
                                      in_=ot)

    if legalize:
        legalize_waits(nc)
    return nc


def pack_core_inputs(c, x, Wq, bq, Wk, bk, Wv, bv, Wo, bo, S=S_FULL):
    """Pack full-model inputs into core c's device tensors."""
    b = c // 2
    hh = c % 2
    hs = slice(hh * 8, hh * 8 + 8)

    def pack_w(W):  # [8, D, DH] -> [128, 4096]: free = chunk*512 + (h*64+dh)
        W2 = np.transpose(W, (1, 0, 2)).reshape(D, 512)      # [d, h*dh]
        return np.ascontiguousarray(
            np.transpose(W2.reshape(8, 128, 512), (1, 0, 2)).reshape(128, 4096)
        )

    def split8(a):
        hi = a.astype(NPF8)
        lo = (a - hi.astype(np.float32)).astype(NPF8)
        return hi, lo

    xT = np.ascontiguousarray(x[b].T)                         # [D, S]
    xh, xl = split8(XSCALE * xT.astype(np.float32))
    wqh, wql = split8(WSCALE * pack_w(Wq[hs]))
    wkh, wkl = split8(WSCALE * pack_w(Wk[hs]))
    wvh, wvl = split8(WSCALE * pack_w(Wv[hs]))
    # Wo rows for this half's features: [512, 1024] -> [128, 4*1024]
    Wr = Wo[hh * 512:(hh + 1) * 512]
    wo = np.ascontiguousarray(
        np.transpose(Wr.reshape(4, 128, 1024), (1, 0, 2)).reshape(128, 4096)
    )
    woh, wol = split8(WSCALE * wo)
    bqk = QSCALE * np.concatenate(
        [bq[hs].reshape(4, 128).T, bk[hs].reshape(4, 128).T], axis=1
    )                                                         # [128, 8]
    bvp = QSCALE * bv[hs].reshape(1, 512)
    bop = (0.5 * bo).reshape(1, 1024)
    NST = S // 128
    vinit = np.zeros((1, NST * 520), dtype=np.float32)
    # ones column = QSCALE so ps_c row 64 = QSCALE*sum(at): its reciprocal
    # normalizes the QSCALE-scaled v in one step.
    vinit[0, 64::65] = QSCALE
    cpk = np.concatenate([
        bvp.ravel().astype(np.float32),
        bop.ravel().astype(np.float32),
    ]).reshape(1, 1536)
    return {
        "vinit": vinit.astype(ml_dtypes.bfloat16),
        "cpk": cpk,
        "ones": np.full((1, 64), CSCALE, dtype=np.float32),
        "xth": xh, "xtl": xl,
        "wqh": wqh, "wql": wql,
        "wkh": wkh, "wkl": wkl,
        "wvh": wvh, "wvl": wvl,
        "woh": woh, "wol": wol,
        "bqk": np.ascontiguousarray(bqk).astype(np.float32),
    }


_NC_CACHE = {}


def _get_nc(S=S_FULL):
    if S not in _NC_CACHE:
        _NC_CACHE[S] = build_nc(S)
    return _NC_CACHE[S]


def kernel(x, Wq, bq, Wk, bk, Wv, bv, Wo, bo, _trace=False):
    x, Wq, bq, Wk, bk, Wv, bv, Wo, bo = (
        np.asarray(a, dtype=np.float32) for a in (x, Wq, bq, Wk, bk, Wv, bv, Wo, bo)
    )
    nc = _get_nc()
    in_maps = [
        pack_core_inputs(c, x, Wq, bq, Wk, bk, Wv, bv, Wo, bo) for c in range(NCORES)
    ]
    res = run_bass_kernel_spmd(nc, in_maps, list(range(NCORES)), trace=_trace)
    out = np.empty((B, S_FULL, D), dtype=np.float32)
    for b in range(B):
        out[b] = res.results[2 * b]["out"] + res.results[2 * b + 1]["out"]
    if _trace:
        kernel.last_results = res
    return out
